# revision 5
# baseline (speedup 1.0000x reference)
"""Bi-directional attention fusion kernel for Trainium2 (8 NeuronCores).

Computes, per batch b (fully data-parallel, one batch per core):
    S       = g @ l.T                                  # [N, N]
    out     = 0.5 * (softmax_rows(S) @ l + softmax_rows(S.T) @ g)

Key algebraic trick: with a *global* stabilization constant c,
    E = exp(S - c)   (layout [g, l])
    F = exp(S.T - c) = E.T  (layout [l, g])
and the two attention terms become plain matmuls where E and F are the
pre-transposed (lhsT) operands directly:
    term1[g, d] = sum_l E[g, l] * l_emb[l, d]  =  (F as lhsT).T @ [l_emb | 1]
    term2[l, d] = sum_g E[g, l] * g_emb[g, d]  =  (E as lhsT).T @ [g_emb | 1]
The appended ones-column yields the softmax normalizers for free.

Schedule per core:
    A: S tiles via fp32r matmuls (1 cyc/row), exp via ACT -> E (bf16, SBUF)
    B: F = E.T via PE transposes (bf16, via PSUM)
    C: two AV matmuls (bf16) + per-row normalize/average epilogue
"""

import numpy as np
import ml_dtypes

import concourse.bass as bass
import concourse.tile as tile
from concourse import bacc, mybir
from concourse.bass_utils import run_bass_kernel_spmd

BF16 = mybir.dt.bfloat16
F32 = mybir.dt.float32
F32R = mybir.dt.float32r

B = 8
N = 2048
D = 768
C_STAB = 116.0  # global softmax shift; logits are N(0, sqrt(768)) -> max ~111

N_CORES = 8
_ts = bass.ts


def build_nc(n=N, d=D, c_stab=C_STAB):
    """Build the per-core Bass program (identical on all cores)."""
    nt = n // 128  # row tiles
    kd = d // 128  # contraction tiles over the embedding dim
    dp1 = d + 1  # ones column appended
    nb = (n + 511) // 512  # 512-wide column chunks of S

    nc = bacc.Bacc(None, target_bir_lowering=False)

    gt_d = nc.dram_tensor("gt", [d, n], F32R, kind="ExternalInput")
    lt_d = nc.dram_tensor("lt", [d, n], F32R, kind="ExternalInput")
    gn_d = nc.dram_tensor("gn", [n, dp1], BF16, kind="ExternalInput")
    ln_d = nc.dram_tensor("ln", [n, dp1], BF16, kind="ExternalInput")
    id_d = nc.dram_tensor("ident", [128, 128], BF16, kind="ExternalInput")
    out_d = nc.dram_tensor("out", [n, d], F32, kind="ExternalOutput")

    gt_r = gt_d[:].rearrange("(k p) n -> p k n", p=128)  # [128, kd, n]
    lt_r = lt_d[:].rearrange("(k p) n -> p k n", p=128)
    gn_r = gn_d[:].rearrange("(i p) d -> p i d", p=128)  # [128, nt, dp1]
    ln_r = ln_d[:].rearrange("(i p) d -> p i d", p=128)

    with tile.TileContext(nc) as tc:
        with (
            tc.tile_pool(name="const", bufs=1) as const_pool,
            tc.tile_pool(name="e", bufs=1) as e_pool,
        ):
            ident = const_pool.tile([128, 128], BF16)
            nc.sync.dma_start(ident[:], id_d[:])
            biasc = const_pool.tile([128, 1], F32)
            nc.vector.memset(biasc[:], -c_stab)
            e_sb = e_pool.tile([128, nt, n], BF16)  # E[g, l]

            # ---- Stage A: E = exp(g @ l.T - c) ----
            with (
                tc.tile_pool(name="a_lt", bufs=1) as lt_pool,
                tc.tile_pool(name="a_gt", bufs=3) as gt_pool,
                tc.tile_pool(name="a_ps", bufs=2, space="PSUM") as ps_a,
            ):
                lt_sb = lt_pool.tile([128, kd, n], F32R)
                for j in range(nb):
                    nc.sync.dma_start(
                        lt_sb[:, :, _ts(j, 512)], lt_r[:, :, _ts(j, 512)]
                    )
                for gi in range(nt):
                    gts = gt_pool.tile([128, kd, 128], F32R)
                    nc.sync.dma_start(gts[:], gt_r[:, :, _ts(gi, 128)])
                    ps = ps_a.tile([128, n], F32)
                    for j in range(nb):
                        for k in range(kd):
                            nc.tensor.matmul(
                                ps[:, _ts(j, 512)],
                                gts[:, k, :],
                                lt_sb[:, k, _ts(j, 512)],
                                start=(k == 0),
                                stop=(k == kd - 1),
                            )
                    for j in range(nb):
                        nc.scalar.activation(
                            e_sb[:, gi, _ts(j, 512)],
                            ps[:, _ts(j, 512)],
                            mybir.ActivationFunctionType.Exp,
                            bias=biasc[:],
                        )

            # ---- Stage B: F = E.T (PE transposes) + stage C input loads ----
            with tc.tile_pool(name="f", bufs=1) as f_pool:
                f_sb = f_pool.tile([128, nt, n], BF16)  # F[l, g]
                with tc.tile_pool(name="c_in", bufs=1) as cin_pool:
                    gn_sb = cin_pool.tile([128, nt, dp1], BF16)
                    ln_sb = cin_pool.tile([128, nt, dp1], BF16)
                    for cc in range(0, nt, 4):
                        nc.sync.dma_start(
                            gn_sb[:, cc : cc + 4, :], gn_r[:, cc : cc + 4, :]
                        )
                        nc.sync.dma_start(
                            ln_sb[:, cc : cc + 4, :], ln_r[:, cc : cc + 4, :]
                        )
                    with tc.tile_pool(name="b_ps", bufs=3, space="PSUM") as ps_b:
                        for lj in range(nt):
                            pb = ps_b.tile([128, n], BF16)
                            for gi in range(nt):
                                nc.tensor.transpose(
                                    pb[:, _ts(gi, 128)],
                                    e_sb[:, gi, _ts(lj, 128)],
                                    ident[:],
                                )
                            half = n // 2
                            nc.scalar.copy(
                                f_sb[:, lj, 0:half], pb[:, 0:half]
                            )
                            nc.vector.tensor_copy(
                                f_sb[:, lj, half:n], pb[:, half:n]
                            )

                    # ---- Stage C: AV matmuls + normalize ----
                    with (
                        tc.tile_pool(name="c_ps", bufs=2, space="PSUM") as ps_c,
                        tc.tile_pool(name="c_out", bufs=3) as out_pool,
                        tc.tile_pool(name="c_tmp", bufs=3) as tmp_pool,
                        tc.tile_pool(name="c_small", bufs=4) as small_pool,
                    ):
                        av_chunks = [(c0, min(c0 + 512, dp1)) for c0 in range(0, dp1, 512)]
                        for i in range(nt):
                            ps1 = ps_c.tile([128, dp1], F32, tag="ps1")
                            ps2 = ps_c.tile([128, dp1], F32, tag="ps2")
                            for k in range(nt):
                                lhs_f = f_sb[:, k, _ts(i, 128)]
                                lhs_e = e_sb[:, k, _ts(i, 128)]
                                st = k == 0
                                sp = k == nt - 1
                                for c0, c1 in av_chunks:
                                    nc.tensor.matmul(
                                        ps1[:, c0:c1], lhs_f, ln_sb[:, k, c0:c1],
                                        start=st, stop=sp,
                                    )
                                for c0, c1 in av_chunks:
                                    nc.tensor.matmul(
                                        ps2[:, c0:c1], lhs_e, gn_sb[:, k, c0:c1],
                                        start=st, stop=sp,
                                    )
                            # epilogue: out = 0.5*(ps1[:, :d]/Z1 + ps2[:, :d]/Z2)
                            r1 = small_pool.tile([128, 1], F32, tag="r1")
                            r2 = small_pool.tile([128, 1], F32, tag="r2")
                            nc.vector.reciprocal(r1[:], ps1[:, d:dp1])
                            nc.vector.reciprocal(r2[:], ps2[:, d:dp1])
                            nc.vector.tensor_scalar_mul(r1[:], r1[:], 0.5)
                            nc.vector.tensor_scalar_mul(r2[:], r2[:], 0.5)
                            t1 = tmp_pool.tile([128, d], F32)
                            nc.scalar.activation(
                                t1[:], ps1[:, 0:d],
                                mybir.ActivationFunctionType.Copy,
                                scale=r1[:],
                            )
                            out_t = out_pool.tile([128, d], F32)
                            nc.vector.scalar_tensor_tensor(
                                out_t[:], ps2[:, 0:d], r2[:], t1[:],
                                op0=mybir.AluOpType.mult,
                                op1=mybir.AluOpType.add,
                            )
                            nc.sync.dma_start(out_d[_ts(i, 128), :], out_t[:])

    nc.compile()
    return nc


_NC_CACHE = {}


def get_nc(n=N, d=D):
    key = (n, d)
    if key not in _NC_CACHE:
        _NC_CACHE[key] = build_nc(n, d)
    return _NC_CACHE[key]


def host_prep(global_embedding, local_embedding):
    """Build the 8 per-core input maps from full [B, N, D] fp32 inputs."""
    g = np.asarray(global_embedding, dtype=np.float32)
    l = np.asarray(local_embedding, dtype=np.float32)
    b, n, d = g.shape
    ident = np.eye(128, dtype=ml_dtypes.bfloat16)
    ones = np.ones((n, 1), np.float32)
    in_maps = []
    for i in range(b):
        in_maps.append(
            {
                "gt": np.ascontiguousarray(g[i].T),
                "lt": np.ascontiguousarray(l[i].T),
                "gn": np.concatenate([g[i], ones], axis=1).astype(
                    ml_dtypes.bfloat16
                ),
                "ln": np.concatenate([l[i], ones], axis=1).astype(
                    ml_dtypes.bfloat16
                ),
                "ident": ident,
            }
        )
    return in_maps


def kernel(global_embedding, local_embedding):
    g = np.asarray(global_embedding, dtype=np.float32)
    b, n, d = g.shape
    nc = get_nc(n, d)
    in_maps = host_prep(global_embedding, local_embedding)
    res = run_bass_kernel_spmd(nc, in_maps, list(range(N_CORES)))
    return np.stack([res.results[i]["out"] for i in range(b)]).astype(np.float32)
